# revision 10
# baseline (speedup 1.0000x reference)
"""Trainium2 Bass kernel for L2P top-k prompt selection (topk_masking).

Reference computation:
    nk  = l2_normalize(K, axis=1)                 # [30, 768]
    sim = l2_normalize(x_query) @ nk.T            # [8192, 30]
    idx = top_k(sim, 5)                           # [8192, 5]
    sel = p[idx]                                  # [8192, 5, 20, 768]
    Ek  = sel[:, :, :10, :].reshape(B, 50, 768)
    Ev  = sel[:, :, 10:, :].reshape(B, 50, 768)
    out = stack([Ek, Ev])                         # [2, 8192, 50, 768]

Strategy (8 cores, data-parallel over batch):
  - query normalization is skipped: top-k ranking is invariant to positive
    per-row scaling of the query.
  - scores = xq @ nk.T on TensorE in fp32 (xq transposed on-chip via
    identity matmuls) — full precision so the selected indices match
    jax.lax.top_k exactly (near-ties matter).
  - top-5 via DVE max8/max_index (ties resolved to lowest index).
  - gather p rows via a single-pass bf16 one-hot matmul. The one-hot is
    exact in bf16 and PSUM accumulates fp32, so the gathered values are
    exactly bf16(p): rel err ~1.1e-3 L2, well inside the 2e-2 gate.
    (bf16 moving data streams at full PE rate; fp16 ran at half rate.)
  - output is staged and written to HBM as bf16 in a contiguous dump
    layout [tile*slot, 128, 15360] (one DMA per (tile, slot), alternating
    the two HWDGE queues sync/scalar); the host reassembles/upcasts to
    the [2, B, 50, 768] fp32 result. This halves HBM write traffic —
    the sole roofline term — vs fp32 output.
  - PSUM tiles pair two 512-col matmul chunks (one bank each) so each
    PSUM->SBUF copy moves 1024 cols, halving copy instruction count.
"""

import sys
import types

import numpy as np

_B = 8192
_DK = 768
_D = 768
_POOL = 30
_PLEN = 20
_TOPK = 5
_NCORES = 8
_BSH = _B // _NCORES          # 1024 batch rows per core
_P = 128
_NTILES = _BSH // _P          # 8 tiles of 128 rows
_ROW = _PLEN * _D             # 15360 floats per selected prompt
_HALF = _ROW // 2             # 7680 (Ek / Ev halves)
_CHUNK = 512
_NCH = _ROW // _CHUNK         # 30 psum chunks per (tile, slot)
_NJ = _NTILES * _TOPK         # 40 (tile, slot) output blocks


def _install_axon_hooks():
    """Make trace=True work under axon (profiling); harmless if absent."""
    if "antenv.axon_hooks" in sys.modules:
        return
    try:
        import trn_agent_boot.trn_boot as _tb

        hook = _tb._ntff_profile_via_ctypes("/opt/axon/libaxon_pjrt.so")
    except Exception:
        hook = None
    m = types.ModuleType("antenv.axon_hooks")
    m.get_axon_ntff_profile_hook = lambda: hook
    m.set_axon_ntff_profile_hook = lambda h: None
    sys.modules["antenv.axon_hooks"] = m


def build_bass():
    import concourse.bacc as bacc
    import concourse.mybir as mybir
    import concourse.tile as tile
    from concourse.masks import make_identity

    f32 = mybir.dt.float32
    bf16 = mybir.dt.bfloat16
    nc = bacc.Bacc(None, target_bir_lowering=False)

    xq_d = nc.dram_tensor("xq", [_BSH, _DK], f32, kind="ExternalInput")
    k_d = nc.dram_tensor("kk", [_POOL, _DK], f32, kind="ExternalInput")
    p_d = nc.dram_tensor("pp", [_POOL, _ROW], bf16, kind="ExternalInput")
    out_d = nc.dram_tensor("out", [_NJ, _P, _ROW], bf16, kind="ExternalOutput")

    with tile.TileContext(nc) as tc:
        with (
            tc.tile_pool(name="const", bufs=1) as cpool,
            tc.tile_pool(name="xq", bufs=2) as xqpool,
            tc.tile_pool(name="xqt", bufs=2) as xqtpool,
            tc.tile_pool(name="topk", bufs=2) as tkpool,
            tc.tile_pool(name="oht", bufs=2) as ohtpool,
            tc.tile_pool(name="stage", bufs=4) as stpool,
            tc.tile_pool(name="ps_small", bufs=1, space="PSUM") as pss,
            tc.tile_pool(name="ps_gather", bufs=2, space="PSUM") as psg,
        ):
            # ---- constants and pool-side tensors ----
            ident = cpool.tile([_P, _P], f32)
            make_identity(nc, ident[:])

            iota_i = cpool.tile([_P, _POOL], mybir.dt.int32)
            nc.gpsimd.iota(iota_i[:], [[1, _POOL]], channel_multiplier=0)
            iota_f = cpool.tile([_P, _POOL], f32)
            nc.vector.tensor_copy(iota_f[:], iota_i[:])

            k_sb = cpool.tile([_POOL, _DK], f32)
            nc.gpsimd.dma_start(out=k_sb[:], in_=k_d[:])

            # p pool, pre-cast to bf16 on the host: one 0.9 MB load
            p_h = cpool.tile([_POOL, _ROW], bf16)
            nc.gpsimd.dma_start(out=p_h[:], in_=p_d[:])

            # ---- normalize K rows: nk = K / ||K|| ----
            nk = cpool.tile([_POOL, _DK], f32)
            ss = cpool.tile([_POOL, 1], f32)
            # nk used as scratch for K^2; ss accumulates the row sums
            nc.scalar.activation(
                nk[:], k_sb[:], mybir.ActivationFunctionType.Square, accum_out=ss[:]
            )
            nrm = cpool.tile([_POOL, 1], f32)
            nc.scalar.activation(nrm[:], ss[:], mybir.ActivationFunctionType.Sqrt)
            inv = cpool.tile([_POOL, 1], f32)
            nc.vector.reciprocal(inv[:], nrm[:])
            nc.vector.tensor_scalar_mul(nk[:], k_sb[:], inv[:])

            # ---- nkT [768, 30] as 6 chunks of [128, 30] ----
            nkt = cpool.tile([_P, 6 * _POOL], f32)
            for j in range(6):
                ps_t = pss.tile([_P, _POOL], f32, space="PSUM")
                nc.tensor.transpose(
                    ps_t[:], nk[:, j * _P : (j + 1) * _P], ident[:_POOL, :_POOL]
                )
                nc.vector.tensor_copy(nkt[:, j * _POOL : (j + 1) * _POOL], ps_t[:])

            # ---- per batch tile ----
            for i in range(_NTILES):
                xq_sb = xqpool.tile([_P, _DK], f32)
                nc.gpsimd.dma_start(out=xq_sb[:], in_=xq_d[i * _P : (i + 1) * _P, :])

                # transpose xq tile -> xqT chunks [128f, 128b]
                xqt = xqtpool.tile([_P, _DK], f32)
                for j in range(6):
                    ps_t = pss.tile([_P, _P], f32, space="PSUM")
                    nc.tensor.transpose(
                        ps_t[:], xq_sb[:, j * _P : (j + 1) * _P], ident[:]
                    )
                    nc.scalar.copy(xqt[:, j * _P : (j + 1) * _P], ps_t[:])

                # scores [128b, 30] = sum_j xqT_j.T @ nkT_j
                ps_sc = pss.tile([_P, _POOL], f32, space="PSUM")
                for j in range(6):
                    nc.tensor.matmul(
                        ps_sc[:],
                        lhsT=xqt[:, j * _P : (j + 1) * _P],
                        rhs=nkt[:, j * _POOL : (j + 1) * _POOL],
                        start=(j == 0),
                        stop=(j == 5),
                    )
                sc = tkpool.tile([_P, _POOL], f32)
                nc.vector.tensor_copy(sc[:], ps_sc[:])

                # top-5 indices (ties -> lowest index, like jax.lax.top_k)
                mx = tkpool.tile([_P, 8], f32)
                mi = tkpool.tile([_P, 8], mybir.dt.uint32)
                nc.vector.max(mx[:], sc[:])
                nc.vector.max_index(mi[:], mx[:], sc[:])
                mif = tkpool.tile([_P, 8], f32)
                nc.vector.tensor_copy(mif[:], mi[:])

                # one-hots [128, 30] -> transposed [30, 128] bf16 for matmul lhsT
                oht = ohtpool.tile([_POOL, _TOPK * _P], bf16)
                for t in range(_TOPK):
                    oh = tkpool.tile([_P, _POOL], f32)
                    nc.vector.tensor_tensor(
                        out=oh[:],
                        in0=iota_f[:],
                        in1=mif[:, t : t + 1].to_broadcast([_P, _POOL]),
                        op=mybir.AluOpType.is_equal,
                    )
                    ps_o = pss.tile([_POOL, _P], f32, space="PSUM")
                    nc.tensor.transpose(ps_o[:], oh[:], ident[:])
                    nc.vector.tensor_copy(oht[:, t * _P : (t + 1) * _P], ps_o[:])

                # gather: sel[b] = p[idx[b,t]] via single-pass bf16 one-hot
                # matmul; stage bf16 and dump with one DMA per (tile, slot),
                # alternating the two HWDGE queues. Each PSUM tile spans two
                # banks; two 512-col matmuls fill it (one bank each) and a
                # single 1024-col copy drains it.
                for t in range(_TOPK):
                    st = stpool.tile([_P, _ROW], bf16, tag="st")
                    for g in range(_NCH // 2):
                        ps_g = psg.tile([_P, 2 * _CHUNK], f32, space="PSUM")
                        for h in range(2):
                            c = 2 * g + h
                            nc.tensor.matmul(
                                ps_g[:, h * _CHUNK : (h + 1) * _CHUNK],
                                lhsT=oht[:, t * _P : (t + 1) * _P],
                                rhs=p_h[:, c * _CHUNK : (c + 1) * _CHUNK],
                                start=True,
                                stop=True,
                            )
                        dst = st[:, 2 * g * _CHUNK : 2 * (g + 1) * _CHUNK]
                        if g % 2 == 0:
                            nc.vector.tensor_copy(dst, ps_g[:])
                        else:
                            nc.scalar.copy(dst, ps_g[:])
                    jidx = i * _TOPK + t
                    eng = nc.sync if (jidx % 2 == 0) else nc.scalar
                    eng.dma_start(out=out_d[jidx, :, :], in_=st[:])

    nc.compile()
    return nc


_NC_CACHE = None


def _get_nc():
    global _NC_CACHE
    if _NC_CACHE is None:
        _install_axon_hooks()
        _NC_CACHE = build_bass()
    return _NC_CACHE


def kernel(x_query, x, K, p, layer_id, trace=False, tmpdir=None):
    from concourse.bass_utils import run_bass_kernel_spmd

    nc = _get_nc()

    import ml_dtypes

    x_query = np.ascontiguousarray(np.asarray(x_query, dtype=np.float32))
    K = np.ascontiguousarray(np.asarray(K, dtype=np.float32))
    p2 = np.ascontiguousarray(
        np.asarray(p, dtype=np.float32)
        .reshape(_POOL, _ROW)
        .astype(ml_dtypes.bfloat16)
    )

    in_maps = []
    for c in range(_NCORES):
        in_maps.append(
            {
                "xq": x_query[c * _BSH : (c + 1) * _BSH],
                "kk": K,
                "pp": p2,
            }
        )

    kw = {}
    if trace:
        import concourse.bass_utils as bass_utils

        bass_utils.upload_artifacts = lambda d: d
        kw = {"trace": True, "tmpdir": tmpdir}
    res = run_bass_kernel_spmd(nc, in_maps, core_ids=list(range(_NCORES)), **kw)

    half = _PLEN // 2
    out = np.empty((2, _B, _TOPK * half, _D), dtype=np.float32)
    for c in range(_NCORES):
        arr = res.results[c]["out"].reshape(_NTILES, _TOPK, _P, 2, _HALF)
        # [tile, slot, row, half, elem] -> [half, tile, row, slot, elem]
        shard = arr.transpose(3, 0, 2, 1, 4).reshape(2, _BSH, _TOPK * half, _D)
        out[:, c * _BSH : (c + 1) * _BSH] = shard
    if trace:
        return out, res
    return out


# revision 11
# speedup vs baseline: 1.1897x; 1.1897x over previous
"""Trainium2 Bass kernel for L2P top-k prompt selection (topk_masking).

Reference computation:
    nk  = l2_normalize(K, axis=1)                 # [30, 768]
    sim = l2_normalize(x_query) @ nk.T            # [8192, 30]
    idx = top_k(sim, 5)                           # [8192, 5]
    sel = p[idx]                                  # [8192, 5, 20, 768]
    Ek  = sel[:, :, :10, :].reshape(B, 50, 768)
    Ev  = sel[:, :, 10:, :].reshape(B, 50, 768)
    out = stack([Ek, Ev])                         # [2, 8192, 50, 768]

Strategy (8 cores, data-parallel over batch):
  - query normalization is skipped: top-k ranking is invariant to positive
    per-row scaling of the query.
  - scores = xq @ nk.T on TensorE in fp32 (xq transposed on-chip via
    identity matmuls) — full precision so the selected indices match
    jax.lax.top_k exactly (near-ties matter).
  - top-5 via DVE max8/max_index (ties resolved to lowest index).
  - gather p rows via a single-pass fp16 one-hot matmul. The one-hot is
    exact in fp16 and PSUM accumulates fp32, so the gathered values are
    exactly fp16(p): rel err ~2.4e-4 L2, far inside the 2e-2 gate.
  - output is staged and written to HBM as fp16 in a contiguous dump
    layout [tile*slot, 128, 15360] (one DMA per (tile, slot), alternating
    the two HWDGE queues sync/scalar); the host reassembles/upcasts to
    the [2, B, 50, 768] fp32 result. This halves HBM write traffic —
    the sole roofline term — vs fp32 output.
"""

import sys
import types

import numpy as np

_B = 8192
_DK = 768
_D = 768
_POOL = 30
_PLEN = 20
_TOPK = 5
_NCORES = 8
_BSH = _B // _NCORES          # 1024 batch rows per core
_P = 128
_NTILES = _BSH // _P          # 8 tiles of 128 rows
_ROW = _PLEN * _D             # 15360 floats per selected prompt
_HALF = _ROW // 2             # 7680 (Ek / Ev halves)
_CHUNK = 512
_NCH = _ROW // _CHUNK         # 30 psum chunks per (tile, slot)
_NJ = _NTILES * _TOPK         # 40 (tile, slot) output blocks


def _install_axon_hooks():
    """Make trace=True work under axon (profiling); harmless if absent."""
    if "antenv.axon_hooks" in sys.modules:
        return
    try:
        import trn_agent_boot.trn_boot as _tb

        hook = _tb._ntff_profile_via_ctypes("/opt/axon/libaxon_pjrt.so")
    except Exception:
        hook = None
    m = types.ModuleType("antenv.axon_hooks")
    m.get_axon_ntff_profile_hook = lambda: hook
    m.set_axon_ntff_profile_hook = lambda h: None
    sys.modules["antenv.axon_hooks"] = m


def build_bass():
    import concourse.bacc as bacc
    import concourse.mybir as mybir
    import concourse.tile as tile
    from concourse.masks import make_identity

    f32 = mybir.dt.float32
    f16 = mybir.dt.float16
    nc = bacc.Bacc(None, target_bir_lowering=False)

    xq_d = nc.dram_tensor("xq", [_BSH, _DK], f32, kind="ExternalInput")
    k_d = nc.dram_tensor("kk", [_POOL, _DK], f32, kind="ExternalInput")
    p_d = nc.dram_tensor("pp", [_POOL, _ROW], f16, kind="ExternalInput")
    out_d = nc.dram_tensor("out", [_NJ, _P, _ROW], f16, kind="ExternalOutput")

    with tile.TileContext(nc) as tc:
        with (
            tc.tile_pool(name="const", bufs=1) as cpool,
            tc.tile_pool(name="xq", bufs=2) as xqpool,
            tc.tile_pool(name="xqt", bufs=2) as xqtpool,
            tc.tile_pool(name="topk", bufs=2) as tkpool,
            tc.tile_pool(name="oht", bufs=2) as ohtpool,
            tc.tile_pool(name="stage", bufs=4) as stpool,
            tc.tile_pool(name="ps_small", bufs=1, space="PSUM") as pss,
            tc.tile_pool(name="ps_gather", bufs=4, space="PSUM") as psg,
        ):
            # ---- constants and pool-side tensors ----
            ident = cpool.tile([_P, _P], f32)
            make_identity(nc, ident[:])

            iota_i = cpool.tile([_P, _POOL], mybir.dt.int32)
            nc.gpsimd.iota(iota_i[:], [[1, _POOL]], channel_multiplier=0)
            iota_f = cpool.tile([_P, _POOL], f32)
            nc.vector.tensor_copy(iota_f[:], iota_i[:])

            k_sb = cpool.tile([_POOL, _DK], f32)
            nc.gpsimd.dma_start(out=k_sb[:], in_=k_d[:])

            # p pool, pre-cast to fp16 on the host: one 0.9 MB load
            p_h = cpool.tile([_POOL, _ROW], f16)
            nc.gpsimd.dma_start(out=p_h[:], in_=p_d[:])

            # ---- normalize K rows: nk = K / ||K|| ----
            nk = cpool.tile([_POOL, _DK], f32)
            ss = cpool.tile([_POOL, 1], f32)
            # nk used as scratch for K^2; ss accumulates the row sums
            nc.scalar.activation(
                nk[:], k_sb[:], mybir.ActivationFunctionType.Square, accum_out=ss[:]
            )
            nrm = cpool.tile([_POOL, 1], f32)
            nc.scalar.activation(nrm[:], ss[:], mybir.ActivationFunctionType.Sqrt)
            inv = cpool.tile([_POOL, 1], f32)
            nc.vector.reciprocal(inv[:], nrm[:])
            nc.vector.tensor_scalar_mul(nk[:], k_sb[:], inv[:])

            # ---- nkT [768, 30] as 6 chunks of [128, 30] ----
            nkt = cpool.tile([_P, 6 * _POOL], f32)
            for j in range(6):
                ps_t = pss.tile([_P, _POOL], f32, space="PSUM")
                nc.tensor.transpose(
                    ps_t[:], nk[:, j * _P : (j + 1) * _P], ident[:_POOL, :_POOL]
                )
                nc.vector.tensor_copy(nkt[:, j * _POOL : (j + 1) * _POOL], ps_t[:])

            # ---- per batch tile ----
            for i in range(_NTILES):
                xq_sb = xqpool.tile([_P, _DK], f32)
                nc.gpsimd.dma_start(out=xq_sb[:], in_=xq_d[i * _P : (i + 1) * _P, :])

                # transpose xq tile -> xqT chunks [128f, 128b]
                xqt = xqtpool.tile([_P, _DK], f32)
                for j in range(6):
                    ps_t = pss.tile([_P, _P], f32, space="PSUM")
                    nc.tensor.transpose(
                        ps_t[:], xq_sb[:, j * _P : (j + 1) * _P], ident[:]
                    )
                    nc.scalar.copy(xqt[:, j * _P : (j + 1) * _P], ps_t[:])

                # scores [128b, 30] = sum_j xqT_j.T @ nkT_j
                ps_sc = pss.tile([_P, _POOL], f32, space="PSUM")
                for j in range(6):
                    nc.tensor.matmul(
                        ps_sc[:],
                        lhsT=xqt[:, j * _P : (j + 1) * _P],
                        rhs=nkt[:, j * _POOL : (j + 1) * _POOL],
                        start=(j == 0),
                        stop=(j == 5),
                    )
                sc = tkpool.tile([_P, _POOL], f32)
                nc.vector.tensor_copy(sc[:], ps_sc[:])

                # top-5 indices (ties -> lowest index, like jax.lax.top_k)
                mx = tkpool.tile([_P, 8], f32)
                mi = tkpool.tile([_P, 8], mybir.dt.uint32)
                nc.vector.max(mx[:], sc[:])
                nc.vector.max_index(mi[:], mx[:], sc[:])
                mif = tkpool.tile([_P, 8], f32)
                nc.vector.tensor_copy(mif[:], mi[:])

                # one-hots [128, 30] -> transposed [30, 128] fp16 for matmul lhsT
                oht = ohtpool.tile([_POOL, _TOPK * _P], f16)
                for t in range(_TOPK):
                    oh = tkpool.tile([_P, _POOL], f32)
                    nc.vector.tensor_tensor(
                        out=oh[:],
                        in0=iota_f[:],
                        in1=mif[:, t : t + 1].to_broadcast([_P, _POOL]),
                        op=mybir.AluOpType.is_equal,
                    )
                    ps_o = pss.tile([_POOL, _P], f32, space="PSUM")
                    nc.tensor.transpose(ps_o[:], oh[:], ident[:])
                    nc.vector.tensor_copy(oht[:, t * _P : (t + 1) * _P], ps_o[:])

                # gather: sel[b] = p[idx[b,t]] via single-pass fp16 one-hot
                # matmul; stage fp16 and dump with one DMA per (tile, slot),
                # alternating the two HWDGE queues.
                for t in range(_TOPK):
                    st = stpool.tile([_P, _ROW], f16, tag="st")
                    for c in range(_NCH):
                        ps_g = psg.tile([_P, _CHUNK], f32, space="PSUM")
                        nc.tensor.matmul(
                            ps_g[:],
                            lhsT=oht[:, t * _P : (t + 1) * _P],
                            rhs=p_h[:, c * _CHUNK : (c + 1) * _CHUNK],
                            start=True,
                            stop=True,
                        )
                        dst = st[:, c * _CHUNK : (c + 1) * _CHUNK]
                        # 3:2 DVE/ACT split keeps both copy engines ahead
                        # of the matmul stream without starving either
                        if c % 5 < 3:
                            nc.vector.tensor_copy(dst, ps_g[:])
                        else:
                            nc.scalar.copy(dst, ps_g[:])
                    jidx = i * _TOPK + t
                    eng = nc.sync if (jidx % 2 == 0) else nc.scalar
                    eng.dma_start(out=out_d[jidx, :, :], in_=st[:])

    nc.compile()
    return nc


_NC_CACHE = None


def _get_nc():
    global _NC_CACHE
    if _NC_CACHE is None:
        _install_axon_hooks()
        _NC_CACHE = build_bass()
    return _NC_CACHE


def kernel(x_query, x, K, p, layer_id, trace=False, tmpdir=None):
    from concourse.bass_utils import run_bass_kernel_spmd

    nc = _get_nc()

    x_query = np.ascontiguousarray(np.asarray(x_query, dtype=np.float32))
    K = np.ascontiguousarray(np.asarray(K, dtype=np.float32))
    p2 = np.ascontiguousarray(
        np.asarray(p, dtype=np.float32).reshape(_POOL, _ROW).astype(np.float16)
    )

    in_maps = []
    for c in range(_NCORES):
        in_maps.append(
            {
                "xq": x_query[c * _BSH : (c + 1) * _BSH],
                "kk": K,
                "pp": p2,
            }
        )

    kw = {}
    if trace:
        import concourse.bass_utils as bass_utils

        bass_utils.upload_artifacts = lambda d: d
        kw = {"trace": True, "tmpdir": tmpdir}
    res = run_bass_kernel_spmd(nc, in_maps, core_ids=list(range(_NCORES)), **kw)

    half = _PLEN // 2
    out = np.empty((2, _B, _TOPK * half, _D), dtype=np.float32)
    for c in range(_NCORES):
        arr = res.results[c]["out"].reshape(_NTILES, _TOPK, _P, 2, _HALF)
        # [tile, slot, row, half, elem] -> [half, tile, row, slot, elem]
        shard = arr.transpose(3, 0, 2, 1, 4).reshape(2, _BSH, _TOPK * half, _D)
        out[:, c * _BSH : (c + 1) * _BSH] = shard
    if trace:
        return out, res
    return out


# revision 17
# speedup vs baseline: 1.2069x; 1.0144x over previous
"""Trainium2 Bass kernel for L2P top-k prompt selection (topk_masking).

Reference computation:
    nk  = l2_normalize(K, axis=1)                 # [30, 768]
    sim = l2_normalize(x_query) @ nk.T            # [8192, 30]
    idx = top_k(sim, 5)                           # [8192, 5]
    sel = p[idx]                                  # [8192, 5, 20, 768]
    Ek  = sel[:, :, :10, :].reshape(B, 50, 768)
    Ev  = sel[:, :, 10:, :].reshape(B, 50, 768)
    out = stack([Ek, Ev])                         # [2, 8192, 50, 768]

Strategy (8 cores, data-parallel over batch):
  - query normalization is skipped: top-k ranking is invariant to positive
    per-row scaling of the query.
  - scores = xq @ nk.T on TensorE in fp32 (xq transposed on-chip via
    identity matmuls) — full precision so the selected indices match
    jax.lax.top_k exactly (near-ties matter).
  - top-5 via DVE max8/max_index (ties resolved to lowest index).
  - gather p rows via a single-pass fp16 one-hot matmul. The one-hot is
    exact in fp16 and PSUM accumulates fp32, so the gathered values are
    exactly fp16(p): rel err ~2.4e-4 L2, far inside the 2e-2 gate.
  - output is staged and written to HBM as fp16 in a contiguous dump
    layout [tile*slot, 128, 15360] (one DMA per (tile, slot), alternating
    the two HWDGE queues sync/scalar); the host reassembles/upcasts to
    the [2, B, 50, 768] fp32 result. This halves HBM write traffic —
    the sole roofline term — vs fp32 output.
"""

import sys
import types

import numpy as np

_B = 8192
_DK = 768
_D = 768
_POOL = 30
_PLEN = 20
_TOPK = 5
_NCORES = 8
_BSH = _B // _NCORES          # 1024 batch rows per core
_P = 128
_NTILES = _BSH // _P          # 8 tiles of 128 rows
_ROW = _PLEN * _D             # 15360 floats per selected prompt
_HALF = _ROW // 2             # 7680 (Ek / Ev halves)
_CHUNK = 512
_NCH = _ROW // _CHUNK         # 30 psum chunks per (tile, slot)
_NJ = _NTILES * _TOPK         # 40 (tile, slot) output blocks


def _install_axon_hooks():
    """Make trace=True work under axon (profiling); harmless if absent."""
    if "antenv.axon_hooks" in sys.modules:
        return
    try:
        import trn_agent_boot.trn_boot as _tb

        hook = _tb._ntff_profile_via_ctypes("/opt/axon/libaxon_pjrt.so")
    except Exception:
        hook = None
    m = types.ModuleType("antenv.axon_hooks")
    m.get_axon_ntff_profile_hook = lambda: hook
    m.set_axon_ntff_profile_hook = lambda h: None
    sys.modules["antenv.axon_hooks"] = m


def build_bass():
    import concourse.bacc as bacc
    import concourse.mybir as mybir
    import concourse.tile as tile
    from concourse.masks import make_identity

    f32 = mybir.dt.float32
    f16 = mybir.dt.float16
    nc = bacc.Bacc(None, target_bir_lowering=False)

    xq_d = nc.dram_tensor("xq", [_BSH, _DK], f32, kind="ExternalInput")
    k_d = nc.dram_tensor("kk", [_POOL, _DK], f32, kind="ExternalInput")
    p_d = nc.dram_tensor("pp", [_POOL, _ROW], f16, kind="ExternalInput")
    out_d = nc.dram_tensor("out", [_NJ, _P, _ROW], f16, kind="ExternalOutput")

    with tile.TileContext(nc) as tc:
        with (
            tc.tile_pool(name="const", bufs=1) as cpool,
            tc.tile_pool(name="xq", bufs=2) as xqpool,
            tc.tile_pool(name="xqt", bufs=2) as xqtpool,
            tc.tile_pool(name="topk", bufs=2) as tkpool,
            tc.tile_pool(name="oht", bufs=2) as ohtpool,
            tc.tile_pool(name="stage", bufs=4) as stpool,
            tc.tile_pool(name="ps_small", bufs=1, space="PSUM") as pss,
            tc.tile_pool(name="ps_gather", bufs=3, space="PSUM") as psg,
        ):
            # ---- constants and pool-side tensors ----
            ident = cpool.tile([_P, _P], f32)
            make_identity(nc, ident[:])

            iota_i = cpool.tile([_P, _POOL], mybir.dt.int32)
            nc.gpsimd.iota(iota_i[:], [[1, _POOL]], channel_multiplier=0)
            iota_f = cpool.tile([_P, _POOL], f32)
            nc.vector.tensor_copy(iota_f[:], iota_i[:])

            k_sb = cpool.tile([_POOL, _DK], f32)
            nc.gpsimd.dma_start(out=k_sb[:], in_=k_d[:])

            # p pool, pre-cast to fp16 on the host: one 0.9 MB load
            p_h = cpool.tile([_POOL, _ROW], f16)
            nc.gpsimd.dma_start(out=p_h[:], in_=p_d[:])

            # ---- normalize K rows: nk = K / ||K|| ----
            nk = cpool.tile([_POOL, _DK], f32)
            ss = cpool.tile([_POOL, 1], f32)
            # nk used as scratch for K^2; ss accumulates the row sums
            nc.scalar.activation(
                nk[:], k_sb[:], mybir.ActivationFunctionType.Square, accum_out=ss[:]
            )
            nrm = cpool.tile([_POOL, 1], f32)
            nc.scalar.activation(nrm[:], ss[:], mybir.ActivationFunctionType.Sqrt)
            inv = cpool.tile([_POOL, 1], f32)
            nc.vector.reciprocal(inv[:], nrm[:])
            nc.vector.tensor_scalar_mul(nk[:], k_sb[:], inv[:])

            # ---- nkT [768, 30] as 6 chunks of [128, 30] ----
            # all ps_small tiles share one [128, 128] shape so the pool
            # occupies a single PSUM bank (frees banks for ps_gather)
            nkt = cpool.tile([_P, 6 * _POOL], f32)
            for j in range(6):
                ps_t = pss.tile([_P, _P], f32, space="PSUM", tag="ps")
                nc.tensor.transpose(
                    ps_t[:, :_POOL], nk[:, j * _P : (j + 1) * _P], ident[:_POOL, :_POOL]
                )
                nc.vector.tensor_copy(
                    nkt[:, j * _POOL : (j + 1) * _POOL], ps_t[:, :_POOL]
                )

            # ---- per batch tile ----
            for i in range(_NTILES):
                xq_sb = xqpool.tile([_P, _DK], f32)
                nc.gpsimd.dma_start(out=xq_sb[:], in_=xq_d[i * _P : (i + 1) * _P, :])

                # transpose xq tile -> xqT chunks [128f, 128b]
                xqt = xqtpool.tile([_P, _DK], f32)
                for j in range(6):
                    ps_t = pss.tile([_P, _P], f32, space="PSUM", tag="ps")
                    nc.tensor.transpose(
                        ps_t[:], xq_sb[:, j * _P : (j + 1) * _P], ident[:]
                    )
                    nc.scalar.copy(xqt[:, j * _P : (j + 1) * _P], ps_t[:])

                # scores [128b, 30] = sum_j xqT_j.T @ nkT_j
                ps_sc = pss.tile([_P, _P], f32, space="PSUM", tag="ps")
                for j in range(6):
                    nc.tensor.matmul(
                        ps_sc[:, :_POOL],
                        lhsT=xqt[:, j * _P : (j + 1) * _P],
                        rhs=nkt[:, j * _POOL : (j + 1) * _POOL],
                        start=(j == 0),
                        stop=(j == 5),
                    )
                sc = tkpool.tile([_P, _POOL], f32)
                nc.vector.tensor_copy(sc[:], ps_sc[:, :_POOL])

                # top-5 indices (ties -> lowest index, like jax.lax.top_k)
                mx = tkpool.tile([_P, 8], f32)
                mi = tkpool.tile([_P, 8], mybir.dt.uint32)
                nc.vector.max(mx[:], sc[:])
                nc.vector.max_index(mi[:], mx[:], sc[:])
                mif = tkpool.tile([_P, 8], f32)
                nc.vector.tensor_copy(mif[:], mi[:])

                # one-hots [128, 30] -> transposed [30, 128] fp16 for matmul lhsT
                oht = ohtpool.tile([_POOL, _TOPK * _P], f16)
                for t in range(_TOPK):
                    oh = tkpool.tile([_P, _POOL], f32)
                    nc.vector.tensor_tensor(
                        out=oh[:],
                        in0=iota_f[:],
                        in1=mif[:, t : t + 1].to_broadcast([_P, _POOL]),
                        op=mybir.AluOpType.is_equal,
                    )
                    ps_o = pss.tile([_P, _P], f32, space="PSUM", tag="ps")
                    nc.tensor.transpose(ps_o[:_POOL, :], oh[:], ident[:])
                    nc.vector.tensor_copy(
                        oht[:, t * _P : (t + 1) * _P], ps_o[:_POOL, :]
                    )

                # gather: sel[b] = p[idx[b,t]] via single-pass fp16 one-hot
                # matmul; stage fp16 and dump with one DMA per (tile, slot),
                # alternating the two HWDGE queues.
                for t in range(_TOPK):
                    st = stpool.tile([_P, _ROW], f16, tag="st")
                    # two 512-col matmuls fill a 2-bank psum pair (one bank
                    # each); one 1024-col copy drains it, halving the
                    # per-copy fixed overhead vs per-chunk copies
                    for g in range(_NCH // 2):
                        ps_g = psg.tile([_P, 2 * _CHUNK], f32, space="PSUM")
                        for h in range(2):
                            c = 2 * g + h
                            nc.tensor.matmul(
                                ps_g[:, h * _CHUNK : (h + 1) * _CHUNK],
                                lhsT=oht[:, t * _P : (t + 1) * _P],
                                rhs=p_h[:, c * _CHUNK : (c + 1) * _CHUNK],
                                start=True,
                                stop=True,
                            )
                        dst = st[:, 2 * g * _CHUNK : 2 * (g + 1) * _CHUNK]
                        # ACT gets one extra pair: DVE also runs the top-k
                        # and one-hot chain, so keep its copy share lighter
                        if g % 2 == 0:
                            nc.scalar.copy(dst, ps_g[:])
                        else:
                            nc.vector.tensor_copy(dst, ps_g[:])
                    jidx = i * _TOPK + t
                    eng = nc.sync if (jidx % 2 == 0) else nc.scalar
                    eng.dma_start(out=out_d[jidx, :, :], in_=st[:])

    nc.compile()
    return nc


_NC_CACHE = None


def _get_nc():
    global _NC_CACHE
    if _NC_CACHE is None:
        _install_axon_hooks()
        _NC_CACHE = build_bass()
    return _NC_CACHE


def kernel(x_query, x, K, p, layer_id, trace=False, tmpdir=None):
    from concourse.bass_utils import run_bass_kernel_spmd

    nc = _get_nc()

    x_query = np.ascontiguousarray(np.asarray(x_query, dtype=np.float32))
    K = np.ascontiguousarray(np.asarray(K, dtype=np.float32))
    p2 = np.ascontiguousarray(
        np.asarray(p, dtype=np.float32).reshape(_POOL, _ROW).astype(np.float16)
    )

    in_maps = []
    for c in range(_NCORES):
        in_maps.append(
            {
                "xq": x_query[c * _BSH : (c + 1) * _BSH],
                "kk": K,
                "pp": p2,
            }
        )

    kw = {}
    if trace:
        import concourse.bass_utils as bass_utils

        bass_utils.upload_artifacts = lambda d: d
        kw = {"trace": True, "tmpdir": tmpdir}
    res = run_bass_kernel_spmd(nc, in_maps, core_ids=list(range(_NCORES)), **kw)

    half = _PLEN // 2
    out = np.empty((2, _B, _TOPK * half, _D), dtype=np.float32)
    for c in range(_NCORES):
        arr = res.results[c]["out"].reshape(_NTILES, _TOPK, _P, 2, _HALF)
        # [tile, slot, row, half, elem] -> [half, tile, row, slot, elem]
        shard = arr.transpose(3, 0, 2, 1, 4).reshape(2, _BSH, _TOPK * half, _D)
        out[:, c * _BSH : (c + 1) * _BSH] = shard
    if trace:
        return out, res
    return out


# revision 18
# speedup vs baseline: 1.2081x; 1.0010x over previous
"""Trainium2 Bass kernel for L2P top-k prompt selection (topk_masking).

Reference computation:
    nk  = l2_normalize(K, axis=1)                 # [30, 768]
    sim = l2_normalize(x_query) @ nk.T            # [8192, 30]
    idx = top_k(sim, 5)                           # [8192, 5]
    sel = p[idx]                                  # [8192, 5, 20, 768]
    Ek  = sel[:, :, :10, :].reshape(B, 50, 768)
    Ev  = sel[:, :, 10:, :].reshape(B, 50, 768)
    out = stack([Ek, Ev])                         # [2, 8192, 50, 768]

Strategy (8 cores, data-parallel over batch):
  - query normalization is skipped: top-k ranking is invariant to positive
    per-row scaling of the query.
  - scores = xq @ nk.T on TensorE in fp32 (xq transposed on-chip via
    identity matmuls) — full precision so the selected indices match
    jax.lax.top_k exactly (near-ties matter).
  - top-5 via DVE max8/max_index (ties resolved to lowest index).
  - two phases: ALL tiles' scores/top-k/one-hot prep first, then one
    uninterrupted stream of 1200 one-hot gather matmuls. The PE column
    rate (1 col / 1.2 GHz cycle measured on this part) is the kernel
    floor, so the gather stream must never stall on cross-phase deps.
  - gather p rows via a single-pass fp16 one-hot matmul. The one-hot is
    exact in fp16 and PSUM accumulates fp32, so the gathered values are
    exactly fp16(p): rel err ~1.8e-4 L2, far inside the 2e-2 gate.
  - two 512-col matmuls fill a 2-bank PSUM pair; one 1024-col copy
    (alternating Scalar/Vector) drains it, halving copy overhead.
  - output is staged and written to HBM as fp16 in a contiguous dump
    layout [tile*slot, 128, 15360] (one DMA per (tile, slot), alternating
    the two HWDGE queues sync/scalar); the host reassembles/upcasts to
    the [2, B, 50, 768] fp32 result. This halves HBM write traffic vs
    fp32 output.
"""

import sys
import types

import numpy as np

_B = 8192
_DK = 768
_D = 768
_POOL = 30
_PLEN = 20
_TOPK = 5
_NCORES = 8
_BSH = _B // _NCORES          # 1024 batch rows per core
_P = 128
_NTILES = _BSH // _P          # 8 tiles of 128 rows
_ROW = _PLEN * _D             # 15360 floats per selected prompt
_HALF = _ROW // 2             # 7680 (Ek / Ev halves)
_CHUNK = 512
_NCH = _ROW // _CHUNK         # 30 psum chunks per (tile, slot)
_NJ = _NTILES * _TOPK         # 40 (tile, slot) output blocks


def _install_axon_hooks():
    """Make trace=True work under axon (profiling); harmless if absent."""
    if "antenv.axon_hooks" in sys.modules:
        return
    try:
        import trn_agent_boot.trn_boot as _tb

        hook = _tb._ntff_profile_via_ctypes("/opt/axon/libaxon_pjrt.so")
    except Exception:
        hook = None
    m = types.ModuleType("antenv.axon_hooks")
    m.get_axon_ntff_profile_hook = lambda: hook
    m.set_axon_ntff_profile_hook = lambda h: None
    sys.modules["antenv.axon_hooks"] = m


def build_bass():
    import concourse.bacc as bacc
    import concourse.mybir as mybir
    import concourse.tile as tile
    from concourse.masks import make_identity

    f32 = mybir.dt.float32
    f16 = mybir.dt.float16
    nc = bacc.Bacc(None, target_bir_lowering=False)

    xq_d = nc.dram_tensor("xq", [_BSH, _DK], f32, kind="ExternalInput")
    k_d = nc.dram_tensor("kk", [_POOL, _DK], f32, kind="ExternalInput")
    p_d = nc.dram_tensor("pp", [_POOL, _ROW], f16, kind="ExternalInput")
    out_d = nc.dram_tensor("out", [_NJ, _P, _ROW], f16, kind="ExternalOutput")

    with tile.TileContext(nc) as tc:
        with (
            tc.tile_pool(name="const", bufs=1) as cpool,
            tc.tile_pool(name="xq", bufs=2) as xqpool,
            tc.tile_pool(name="xqt", bufs=2) as xqtpool,
            tc.tile_pool(name="topk", bufs=2) as tkpool,
            tc.tile_pool(name="stage", bufs=4) as stpool,
            tc.tile_pool(name="ps_small", bufs=2, space="PSUM") as pss,
            tc.tile_pool(name="ps_gather", bufs=3, space="PSUM") as psg,
        ):
            # ---- constants and pool-side tensors ----
            ident = cpool.tile([_P, _P], f32)
            make_identity(nc, ident[:])

            iota_i = cpool.tile([_P, _POOL], mybir.dt.int32)
            nc.gpsimd.iota(iota_i[:], [[1, _POOL]], channel_multiplier=0)
            iota_f = cpool.tile([_P, _POOL], f32)
            nc.vector.tensor_copy(iota_f[:], iota_i[:])

            k_sb = cpool.tile([_POOL, _DK], f32)
            nc.gpsimd.dma_start(out=k_sb[:], in_=k_d[:])

            # p pool, pre-cast to fp16 on the host: one 0.9 MB load
            p_h = cpool.tile([_POOL, _ROW], f16)
            nc.gpsimd.dma_start(out=p_h[:], in_=p_d[:])

            # all 40 transposed one-hots [30, 40*128], built in phase A
            ohts = cpool.tile([_POOL, _NJ * _P], f16)

            # ---- normalize K rows: nk = K / ||K|| ----
            nk = cpool.tile([_POOL, _DK], f32)
            ss = cpool.tile([_POOL, 1], f32)
            # nk used as scratch for K^2; ss accumulates the row sums
            nc.scalar.activation(
                nk[:], k_sb[:], mybir.ActivationFunctionType.Square, accum_out=ss[:]
            )
            nrm = cpool.tile([_POOL, 1], f32)
            nc.scalar.activation(nrm[:], ss[:], mybir.ActivationFunctionType.Sqrt)
            inv = cpool.tile([_POOL, 1], f32)
            nc.vector.reciprocal(inv[:], nrm[:])
            nc.vector.tensor_scalar_mul(nk[:], k_sb[:], inv[:])

            # ---- nkT [768, 30] as 6 chunks of [128, 30] ----
            # ps_small tiles share one [128, 128] shape/tag: 2 banks total
            nkt = cpool.tile([_P, 6 * _POOL], f32)
            for j in range(6):
                ps_t = pss.tile([_P, _P], f32, space="PSUM", tag="ps")
                nc.tensor.transpose(
                    ps_t[:, :_POOL], nk[:, j * _P : (j + 1) * _P], ident[:_POOL, :_POOL]
                )
                nc.vector.tensor_copy(
                    nkt[:, j * _POOL : (j + 1) * _POOL], ps_t[:, :_POOL]
                )

            # ---- phase A: scores + top-5 + transposed one-hots, all tiles ----
            for i in range(_NTILES):
                xq_sb = xqpool.tile([_P, _DK], f32)
                nc.gpsimd.dma_start(out=xq_sb[:], in_=xq_d[i * _P : (i + 1) * _P, :])

                # transpose xq tile -> xqT chunks [128f, 128b]
                xqt = xqtpool.tile([_P, _DK], f32)
                for j in range(6):
                    ps_t = pss.tile([_P, _P], f32, space="PSUM", tag="ps")
                    nc.tensor.transpose(
                        ps_t[:], xq_sb[:, j * _P : (j + 1) * _P], ident[:]
                    )
                    dst = xqt[:, j * _P : (j + 1) * _P]
                    if j % 2 == 0:
                        nc.scalar.copy(dst, ps_t[:])
                    else:
                        nc.vector.tensor_copy(dst, ps_t[:])

                # scores [128b, 30] = sum_j xqT_j.T @ nkT_j
                ps_sc = pss.tile([_P, _P], f32, space="PSUM", tag="ps")
                for j in range(6):
                    nc.tensor.matmul(
                        ps_sc[:, :_POOL],
                        lhsT=xqt[:, j * _P : (j + 1) * _P],
                        rhs=nkt[:, j * _POOL : (j + 1) * _POOL],
                        start=(j == 0),
                        stop=(j == 5),
                    )
                sc = tkpool.tile([_P, _POOL], f32)
                nc.vector.tensor_copy(sc[:], ps_sc[:, :_POOL])

                # top-5 indices (ties -> lowest index, like jax.lax.top_k)
                mx = tkpool.tile([_P, 8], f32)
                mi = tkpool.tile([_P, 8], mybir.dt.uint32)
                nc.vector.max(mx[:], sc[:])
                nc.vector.max_index(mi[:], mx[:], sc[:])
                mif = tkpool.tile([_P, 8], f32)
                nc.vector.tensor_copy(mif[:], mi[:])

                # one-hots [128, 30] -> transposed [30, 128] fp16 blocks
                for t in range(_TOPK):
                    oh = tkpool.tile([_P, _POOL], f32)
                    nc.vector.tensor_tensor(
                        out=oh[:],
                        in0=iota_f[:],
                        in1=mif[:, t : t + 1].to_broadcast([_P, _POOL]),
                        op=mybir.AluOpType.is_equal,
                    )
                    ps_o = pss.tile([_P, _P], f32, space="PSUM", tag="ps")
                    nc.tensor.transpose(ps_o[:_POOL, :], oh[:], ident[:])
                    jb = (i * _TOPK + t) * _P
                    if t % 2 == 0:
                        nc.scalar.copy(ohts[:, jb : jb + _P], ps_o[:_POOL, :])
                    else:
                        nc.vector.tensor_copy(ohts[:, jb : jb + _P], ps_o[:_POOL, :])

            # ---- phase B: pure gather stream, 30 matmuls per (tile, slot) ----
            for jidx in range(_NJ):
                st = stpool.tile([_P, _ROW], f16, tag="st")
                for g in range(_NCH // 2):
                    ps_g = psg.tile([_P, 2 * _CHUNK], f32, space="PSUM")
                    for h in range(2):
                        c = 2 * g + h
                        nc.tensor.matmul(
                            ps_g[:, h * _CHUNK : (h + 1) * _CHUNK],
                            lhsT=ohts[:, jidx * _P : (jidx + 1) * _P],
                            rhs=p_h[:, c * _CHUNK : (c + 1) * _CHUNK],
                            start=True,
                            stop=True,
                        )
                    dst = st[:, 2 * g * _CHUNK : 2 * (g + 1) * _CHUNK]
                    if g % 2 == 0:
                        nc.scalar.copy(dst, ps_g[:])
                    else:
                        nc.vector.tensor_copy(dst, ps_g[:])
                eng = nc.sync if (jidx % 2 == 0) else nc.scalar
                eng.dma_start(out=out_d[jidx, :, :], in_=st[:])

    nc.compile()
    return nc


_NC_CACHE = None


def _get_nc():
    global _NC_CACHE
    if _NC_CACHE is None:
        _install_axon_hooks()
        _NC_CACHE = build_bass()
    return _NC_CACHE


def kernel(x_query, x, K, p, layer_id, trace=False, tmpdir=None):
    from concourse.bass_utils import run_bass_kernel_spmd

    nc = _get_nc()

    x_query = np.ascontiguousarray(np.asarray(x_query, dtype=np.float32))
    K = np.ascontiguousarray(np.asarray(K, dtype=np.float32))
    p2 = np.ascontiguousarray(
        np.asarray(p, dtype=np.float32).reshape(_POOL, _ROW).astype(np.float16)
    )

    in_maps = []
    for c in range(_NCORES):
        in_maps.append(
            {
                "xq": x_query[c * _BSH : (c + 1) * _BSH],
                "kk": K,
                "pp": p2,
            }
        )

    kw = {}
    if trace:
        import concourse.bass_utils as bass_utils

        bass_utils.upload_artifacts = lambda d: d
        kw = {"trace": True, "tmpdir": tmpdir}
    res = run_bass_kernel_spmd(nc, in_maps, core_ids=list(range(_NCORES)), **kw)

    half = _PLEN // 2
    out = np.empty((2, _B, _TOPK * half, _D), dtype=np.float32)
    for c in range(_NCORES):
        arr = res.results[c]["out"].reshape(_NTILES, _TOPK, _P, 2, _HALF)
        # [tile, slot, row, half, elem] -> [half, tile, row, slot, elem]
        shard = arr.transpose(3, 0, 2, 1, 4).reshape(2, _BSH, _TOPK * half, _D)
        out[:, c * _BSH : (c + 1) * _BSH] = shard
    if trace:
        return out, res
    return out


# revision 21
# speedup vs baseline: 1.2951x; 1.0720x over previous
"""Trainium2 Bass kernel for L2P top-k prompt selection (topk_masking).

Reference computation:
    nk  = l2_normalize(K, axis=1)                 # [30, 768]
    sim = l2_normalize(x_query) @ nk.T            # [8192, 30]
    idx = top_k(sim, 5)                           # [8192, 5]
    sel = p[idx]                                  # [8192, 5, 20, 768]
    Ek  = sel[:, :, :10, :].reshape(B, 50, 768)
    Ev  = sel[:, :, 10:, :].reshape(B, 50, 768)
    out = stack([Ek, Ev])                         # [2, 8192, 50, 768]

Strategy (8 cores, data-parallel over batch):
  - query normalization is skipped: top-k ranking is invariant to positive
    per-row scaling of the query.
  - scores = xq @ nk.T on TensorE in fp32 (xq transposed on-chip via
    identity matmuls) — full precision so the selected indices match
    jax.lax.top_k exactly (near-ties matter).
  - top-5 via DVE max8/max_index (ties resolved to lowest index).
  - two phases: ALL tiles' scores/top-k/one-hot prep first, then one
    uninterrupted stream of 1200 one-hot gather matmuls. The PE column
    rate (1 col / 1.2 GHz cycle measured on this part) is the kernel
    floor, so the gather stream must never stall on cross-phase deps.
  - gather p rows via a single-pass fp16 one-hot matmul. The one-hot is
    exact in fp16 and PSUM accumulates fp32, so the gathered values are
    exactly fp16(p): rel err ~1.8e-4 L2, far inside the 2e-2 gate.
  - two 512-col matmuls fill a 2-bank PSUM pair; one 1024-col copy
    (alternating Scalar/Vector) drains it, halving copy overhead.
  - output is staged and written to HBM as fp16 in a contiguous dump
    layout [tile*slot, 128, 15360] (one DMA per (tile, slot), alternating
    the two HWDGE queues sync/scalar); the host reassembles/upcasts to
    the [2, B, 50, 768] fp32 result. This halves HBM write traffic vs
    fp32 output.
"""

import sys
import types

import numpy as np

_B = 8192
_DK = 768
_D = 768
_POOL = 30
_PLEN = 20
_TOPK = 5
_NCORES = 8
_BSH = _B // _NCORES          # 1024 batch rows per core
_P = 128
_NTILES = _BSH // _P          # 8 tiles of 128 rows
_ROW = _PLEN * _D             # 15360 floats per selected prompt
_HALF = _ROW // 2             # 7680 (Ek / Ev halves)
_CHUNK = 512
_NCH = _ROW // _CHUNK         # 30 psum chunks per (tile, slot)
_NJ = _NTILES * _TOPK         # 40 (tile, slot) output blocks


def _install_axon_hooks():
    """Make trace=True work under axon (profiling); harmless if absent."""
    if "antenv.axon_hooks" in sys.modules:
        return
    try:
        import trn_agent_boot.trn_boot as _tb

        hook = _tb._ntff_profile_via_ctypes("/opt/axon/libaxon_pjrt.so")
    except Exception:
        hook = None
    m = types.ModuleType("antenv.axon_hooks")
    m.get_axon_ntff_profile_hook = lambda: hook
    m.set_axon_ntff_profile_hook = lambda h: None
    sys.modules["antenv.axon_hooks"] = m


def build_bass():
    import concourse.bacc as bacc
    import concourse.mybir as mybir
    import concourse.tile as tile
    from concourse.masks import make_identity

    f32 = mybir.dt.float32
    f16 = mybir.dt.float16
    nc = bacc.Bacc(None, target_bir_lowering=False)

    xq_d = nc.dram_tensor("xq", [_BSH, _DK], f32, kind="ExternalInput")
    k_d = nc.dram_tensor("kk", [_POOL, _DK], f32, kind="ExternalInput")
    p_d = nc.dram_tensor("pp", [_POOL, _ROW], f16, kind="ExternalInput")
    out_d = nc.dram_tensor("out", [_NJ, _P, _ROW], f16, kind="ExternalOutput")

    with tile.TileContext(nc) as tc:
        with (
            tc.tile_pool(name="const", bufs=1) as cpool,
            tc.tile_pool(name="xq", bufs=2) as xqpool,
            tc.tile_pool(name="xqt", bufs=2) as xqtpool,
            tc.tile_pool(name="topk", bufs=2) as tkpool,
            tc.tile_pool(name="stage", bufs=4) as stpool,
            tc.tile_pool(name="ps_small", bufs=2, space="PSUM") as pss,
            tc.tile_pool(name="ps_gather", bufs=3, space="PSUM") as psg,
        ):
            # ---- constants and pool-side tensors ----
            ident = cpool.tile([_P, _P], f32)
            make_identity(nc, ident[:])

            iota_i = cpool.tile([_P, _POOL], mybir.dt.int32)
            nc.gpsimd.iota(iota_i[:], [[1, _POOL]], channel_multiplier=0)
            iota_f = cpool.tile([_P, _POOL], f32)
            nc.vector.tensor_copy(iota_f[:], iota_i[:])

            k_sb = cpool.tile([_POOL, _DK], f32)
            nc.gpsimd.dma_start(out=k_sb[:], in_=k_d[:])

            # all 40 transposed one-hots [30, 40*128], built in phase A
            ohts = cpool.tile([_POOL, _NJ * _P], f16)

            # K-normalization runs on ACT/DVE while PE does tile 0's xq
            # transposes; nkT lands on PE between those and the first scores
            nk = cpool.tile([_POOL, _DK], f32)
            ss = cpool.tile([_POOL, 1], f32)
            nrm = cpool.tile([_POOL, 1], f32)
            inv = cpool.tile([_POOL, 1], f32)
            nkt = cpool.tile([_P, 6 * _POOL], f32)
            p_h = cpool.tile([_POOL, _ROW], f16)

            # ---- phase A: scores + top-5 + transposed one-hots, all tiles ----
            for i in range(_NTILES):
                xq_sb = xqpool.tile([_P, _DK], f32)
                nc.gpsimd.dma_start(out=xq_sb[:], in_=xq_d[i * _P : (i + 1) * _P, :])

                # transpose xq tile -> xqT chunks [128f, 128b]
                xqt = xqtpool.tile([_P, _DK], f32)
                for j in range(6):
                    ps_t = pss.tile([_P, _P], f32, space="PSUM", tag="ps")
                    nc.tensor.transpose(
                        ps_t[:], xq_sb[:, j * _P : (j + 1) * _P], ident[:]
                    )
                    dst = xqt[:, j * _P : (j + 1) * _P]
                    if j % 2 == 0:
                        nc.scalar.copy(dst, ps_t[:])
                    else:
                        nc.vector.tensor_copy(dst, ps_t[:])

                if i == 0:
                    # normalize K rows: nk = K / ||K|| (ACT/DVE), then
                    # nkT [768, 30] as 6 chunks of [128, 30] on PE
                    nc.scalar.activation(
                        nk[:],
                        k_sb[:],
                        mybir.ActivationFunctionType.Square,
                        accum_out=ss[:],
                    )
                    nc.scalar.activation(
                        nrm[:], ss[:], mybir.ActivationFunctionType.Sqrt
                    )
                    nc.vector.reciprocal(inv[:], nrm[:])
                    nc.vector.tensor_scalar_mul(nk[:], k_sb[:], inv[:])
                    for j in range(6):
                        ps_t = pss.tile([_P, _P], f32, space="PSUM", tag="ps")
                        nc.tensor.transpose(
                            ps_t[:, :_POOL],
                            nk[:, j * _P : (j + 1) * _P],
                            ident[:_POOL, :_POOL],
                        )
                        nc.vector.tensor_copy(
                            nkt[:, j * _POOL : (j + 1) * _POOL], ps_t[:, :_POOL]
                        )
                    # p pool, pre-cast to fp16 on the host; queued after the
                    # first xq tiles so it doesn't delay phase A's start
                    nc.gpsimd.dma_start(out=p_h[:], in_=p_d[:])

                # scores [128b, 30] = sum_j xqT_j.T @ nkT_j
                ps_sc = pss.tile([_P, _P], f32, space="PSUM", tag="ps")
                for j in range(6):
                    nc.tensor.matmul(
                        ps_sc[:, :_POOL],
                        lhsT=xqt[:, j * _P : (j + 1) * _P],
                        rhs=nkt[:, j * _POOL : (j + 1) * _POOL],
                        start=(j == 0),
                        stop=(j == 5),
                    )
                sc = tkpool.tile([_P, _POOL], f32)
                nc.vector.tensor_copy(sc[:], ps_sc[:, :_POOL])

                # top-5 indices (ties -> lowest index, like jax.lax.top_k)
                mx = tkpool.tile([_P, 8], f32)
                mi = tkpool.tile([_P, 8], mybir.dt.uint32)
                nc.vector.max(mx[:], sc[:])
                nc.vector.max_index(mi[:], mx[:], sc[:])
                mif = tkpool.tile([_P, 8], f32)
                nc.vector.tensor_copy(mif[:], mi[:])

                # one-hots [128, 30] -> transposed [30, 128] fp16 blocks
                for t in range(_TOPK):
                    oh = tkpool.tile([_P, _POOL], f32)
                    nc.vector.tensor_tensor(
                        out=oh[:],
                        in0=iota_f[:],
                        in1=mif[:, t : t + 1].to_broadcast([_P, _POOL]),
                        op=mybir.AluOpType.is_equal,
                    )
                    ps_o = pss.tile([_P, _P], f32, space="PSUM", tag="ps")
                    nc.tensor.transpose(ps_o[:_POOL, :], oh[:], ident[:])
                    jb = (i * _TOPK + t) * _P
                    if t % 2 == 0:
                        nc.scalar.copy(ohts[:, jb : jb + _P], ps_o[:_POOL, :])
                    else:
                        nc.vector.tensor_copy(ohts[:, jb : jb + _P], ps_o[:_POOL, :])

            # ---- phase B: pure gather stream, 30 matmuls per (tile, slot) ----
            for jidx in range(_NJ):
                st = stpool.tile([_P, _ROW], f16, tag="st")
                for g in range(_NCH // 2):
                    ps_g = psg.tile([_P, 2 * _CHUNK], f32, space="PSUM")
                    for h in range(2):
                        c = 2 * g + h
                        nc.tensor.matmul(
                            ps_g[:, h * _CHUNK : (h + 1) * _CHUNK],
                            lhsT=ohts[:, jidx * _P : (jidx + 1) * _P],
                            rhs=p_h[:, c * _CHUNK : (c + 1) * _CHUNK],
                            start=True,
                            stop=True,
                        )
                    dst = st[:, 2 * g * _CHUNK : 2 * (g + 1) * _CHUNK]
                    if g % 2 == 0:
                        nc.scalar.copy(dst, ps_g[:])
                    else:
                        nc.vector.tensor_copy(dst, ps_g[:])
                # all output DMAs issue from the otherwise-idle Sync engine:
                # one HWDGE ring still spreads across all 16 SDMA engines,
                # and Scalar never stalls its copy stream on a DMA issue
                nc.sync.dma_start(out=out_d[jidx, :, :], in_=st[:])

    nc.compile()
    return nc


_NC_CACHE = None


def _get_nc():
    global _NC_CACHE
    if _NC_CACHE is None:
        _install_axon_hooks()
        _NC_CACHE = build_bass()
    return _NC_CACHE


def kernel(x_query, x, K, p, layer_id, trace=False, tmpdir=None):
    from concourse.bass_utils import run_bass_kernel_spmd

    nc = _get_nc()

    x_query = np.ascontiguousarray(np.asarray(x_query, dtype=np.float32))
    K = np.ascontiguousarray(np.asarray(K, dtype=np.float32))
    p2 = np.ascontiguousarray(
        np.asarray(p, dtype=np.float32).reshape(_POOL, _ROW).astype(np.float16)
    )

    in_maps = []
    for c in range(_NCORES):
        in_maps.append(
            {
                "xq": x_query[c * _BSH : (c + 1) * _BSH],
                "kk": K,
                "pp": p2,
            }
        )

    kw = {}
    if trace:
        import concourse.bass_utils as bass_utils

        bass_utils.upload_artifacts = lambda d: d
        kw = {"trace": True, "tmpdir": tmpdir}
    res = run_bass_kernel_spmd(nc, in_maps, core_ids=list(range(_NCORES)), **kw)

    half = _PLEN // 2
    out = np.empty((2, _B, _TOPK * half, _D), dtype=np.float32)
    for c in range(_NCORES):
        arr = res.results[c]["out"].reshape(_NTILES, _TOPK, _P, 2, _HALF)
        # [tile, slot, row, half, elem] -> [half, tile, row, slot, elem]
        shard = arr.transpose(3, 0, 2, 1, 4).reshape(2, _BSH, _TOPK * half, _D)
        out[:, c * _BSH : (c + 1) * _BSH] = shard
    if trace:
        return out, res
    return out


# revision 22
# speedup vs baseline: 1.2990x; 1.0030x over previous
"""Trainium2 Bass kernel for L2P top-k prompt selection (topk_masking).

Reference computation:
    nk  = l2_normalize(K, axis=1)                 # [30, 768]
    sim = l2_normalize(x_query) @ nk.T            # [8192, 30]
    idx = top_k(sim, 5)                           # [8192, 5]
    sel = p[idx]                                  # [8192, 5, 20, 768]
    Ek  = sel[:, :, :10, :].reshape(B, 50, 768)
    Ev  = sel[:, :, 10:, :].reshape(B, 50, 768)
    out = stack([Ek, Ev])                         # [2, 8192, 50, 768]

Strategy (8 cores, data-parallel over batch):
  - query normalization is skipped: top-k ranking is invariant to positive
    per-row scaling of the query.
  - scores = xq @ nk.T on TensorE in fp32 (xq transposed on-chip via
    identity matmuls) — full precision so the selected indices match
    jax.lax.top_k exactly (near-ties matter).
  - top-5 via DVE max8/max_index (ties resolved to lowest index).
  - two phases: ALL tiles' scores/top-k/one-hot prep first, then one
    uninterrupted stream of 1200 one-hot gather matmuls. The PE column
    rate (1 col / 1.2 GHz cycle measured on this part) is the kernel
    floor, so the gather stream must never stall on cross-phase deps.
  - gather p rows via a single-pass fp16 one-hot matmul. The one-hot is
    exact in fp16 and PSUM accumulates fp32, so the gathered values are
    exactly fp16(p): rel err ~1.8e-4 L2, far inside the 2e-2 gate.
  - two 512-col matmuls fill a 2-bank PSUM pair; one 1024-col copy
    (alternating Scalar/Vector) drains it, halving copy overhead.
  - output is staged and written to HBM as fp16 in a contiguous dump
    layout [tile*slot, 128, 15360] (one DMA per (tile, slot), alternating
    the two HWDGE queues sync/scalar); the host reassembles/upcasts to
    the [2, B, 50, 768] fp32 result. This halves HBM write traffic vs
    fp32 output.
"""

import sys
import types

import numpy as np

_B = 8192
_DK = 768
_D = 768
_POOL = 30
_PLEN = 20
_TOPK = 5
_NCORES = 8
_BSH = _B // _NCORES          # 1024 batch rows per core
_P = 128
_NTILES = _BSH // _P          # 8 tiles of 128 rows
_ROW = _PLEN * _D             # 15360 floats per selected prompt
_HALF = _ROW // 2             # 7680 (Ek / Ev halves)
_CHUNK = 512
_NCH = _ROW // _CHUNK         # 30 psum chunks per (tile, slot)
_NJ = _NTILES * _TOPK         # 40 (tile, slot) output blocks


def _install_axon_hooks():
    """Make trace=True work under axon (profiling); harmless if absent."""
    if "antenv.axon_hooks" in sys.modules:
        return
    try:
        import trn_agent_boot.trn_boot as _tb

        hook = _tb._ntff_profile_via_ctypes("/opt/axon/libaxon_pjrt.so")
    except Exception:
        hook = None
    m = types.ModuleType("antenv.axon_hooks")
    m.get_axon_ntff_profile_hook = lambda: hook
    m.set_axon_ntff_profile_hook = lambda h: None
    sys.modules["antenv.axon_hooks"] = m


def build_bass():
    import concourse.bacc as bacc
    import concourse.mybir as mybir
    import concourse.tile as tile
    from concourse.masks import make_identity

    f32 = mybir.dt.float32
    f16 = mybir.dt.float16
    nc = bacc.Bacc(None, target_bir_lowering=False)

    xq_d = nc.dram_tensor("xq", [_BSH, _DK], f32, kind="ExternalInput")
    k_d = nc.dram_tensor("kk", [_POOL, _DK], f32, kind="ExternalInput")
    p_d = nc.dram_tensor("pp", [_POOL, _ROW], f16, kind="ExternalInput")
    out_d = nc.dram_tensor("out", [_NJ, _P, _ROW], f16, kind="ExternalOutput")

    with tile.TileContext(nc) as tc:
        with (
            tc.tile_pool(name="const", bufs=1) as cpool,
            tc.tile_pool(name="xq", bufs=2) as xqpool,
            tc.tile_pool(name="xqt", bufs=2) as xqtpool,
            tc.tile_pool(name="topk", bufs=2) as tkpool,
            tc.tile_pool(name="stage", bufs=4) as stpool,
            tc.tile_pool(name="ps_small", bufs=2, space="PSUM") as pss,
            tc.tile_pool(name="ps_gather", bufs=3, space="PSUM") as psg,
        ):
            # ---- constants and pool-side tensors ----
            ident = cpool.tile([_P, _P], f32)
            make_identity(nc, ident[:])

            iota_i = cpool.tile([_P, _POOL], mybir.dt.int32)
            nc.gpsimd.iota(iota_i[:], [[1, _POOL]], channel_multiplier=0)
            iota_f = cpool.tile([_P, _POOL], f32)
            nc.vector.tensor_copy(iota_f[:], iota_i[:])

            k_sb = cpool.tile([_POOL, _DK], f32)
            nc.gpsimd.dma_start(out=k_sb[:], in_=k_d[:])

            # all 40 transposed one-hots [30, 40*128], built in phase A
            ohts = cpool.tile([_POOL, _NJ * _P], f16)

            # K-normalization runs on ACT/DVE while PE does tile 0's xq
            # transposes; nkT lands on PE between those and the first scores
            nk = cpool.tile([_POOL, _DK], f32)
            ss = cpool.tile([_POOL, 1], f32)
            nrm = cpool.tile([_POOL, 1], f32)
            inv = cpool.tile([_POOL, 1], f32)
            nkt = cpool.tile([_P, 6 * _POOL], f32)
            p_h = cpool.tile([_POOL, _ROW], f16)

            # ---- phase A: scores + top-5 + transposed one-hots, all tiles ----
            for i in range(_NTILES):
                xq_sb = xqpool.tile([_P, _DK], f32)
                nc.gpsimd.dma_start(out=xq_sb[:], in_=xq_d[i * _P : (i + 1) * _P, :])

                # transpose xq tile -> xqT chunks [128f, 128b]
                xqt = xqtpool.tile([_P, _DK], f32)
                for j in range(6):
                    ps_t = pss.tile([_P, _P], f32, space="PSUM", tag="ps")
                    nc.tensor.transpose(
                        ps_t[:], xq_sb[:, j * _P : (j + 1) * _P], ident[:]
                    )
                    dst = xqt[:, j * _P : (j + 1) * _P]
                    if j % 2 == 0:
                        nc.scalar.copy(dst, ps_t[:])
                    else:
                        nc.vector.tensor_copy(dst, ps_t[:])

                if i == 0:
                    # normalize K rows: nk = K / ||K|| (ACT/DVE), then
                    # nkT [768, 30] as 6 chunks of [128, 30] on PE
                    nc.scalar.activation(
                        nk[:],
                        k_sb[:],
                        mybir.ActivationFunctionType.Square,
                        accum_out=ss[:],
                    )
                    nc.scalar.activation(
                        nrm[:], ss[:], mybir.ActivationFunctionType.Sqrt
                    )
                    nc.vector.reciprocal(inv[:], nrm[:])
                    nc.vector.tensor_scalar_mul(nk[:], k_sb[:], inv[:])
                    for j in range(6):
                        ps_t = pss.tile([_P, _P], f32, space="PSUM", tag="ps")
                        nc.tensor.transpose(
                            ps_t[:, :_POOL],
                            nk[:, j * _P : (j + 1) * _P],
                            ident[:_POOL, :_POOL],
                        )
                        nc.vector.tensor_copy(
                            nkt[:, j * _POOL : (j + 1) * _POOL], ps_t[:, :_POOL]
                        )
                    # p pool, pre-cast to fp16 on the host; queued after the
                    # first xq tiles so it doesn't delay phase A's start
                    nc.gpsimd.dma_start(out=p_h[:], in_=p_d[:])

                # scores [128b, 30] = sum_j xqT_j.T @ nkT_j
                ps_sc = pss.tile([_P, _P], f32, space="PSUM", tag="ps")
                for j in range(6):
                    nc.tensor.matmul(
                        ps_sc[:, :_POOL],
                        lhsT=xqt[:, j * _P : (j + 1) * _P],
                        rhs=nkt[:, j * _POOL : (j + 1) * _POOL],
                        start=(j == 0),
                        stop=(j == 5),
                    )
                sc = tkpool.tile([_P, _POOL], f32)
                nc.vector.tensor_copy(sc[:], ps_sc[:, :_POOL])

                # top-5 indices (ties -> lowest index, like jax.lax.top_k)
                mx = tkpool.tile([_P, 8], f32)
                mi = tkpool.tile([_P, 8], mybir.dt.uint32)
                nc.vector.max(mx[:], sc[:])
                nc.vector.max_index(mi[:], mx[:], sc[:])
                mif = tkpool.tile([_P, 8], f32)
                nc.vector.tensor_copy(mif[:], mi[:])

                # one-hots [128, 30] -> transposed [30, 128] fp16 blocks
                for t in range(_TOPK):
                    oh = tkpool.tile([_P, _POOL], f32)
                    nc.vector.tensor_tensor(
                        out=oh[:],
                        in0=iota_f[:],
                        in1=mif[:, t : t + 1].to_broadcast([_P, _POOL]),
                        op=mybir.AluOpType.is_equal,
                    )
                    ps_o = pss.tile([_P, _P], f32, space="PSUM", tag="ps")
                    nc.tensor.transpose(ps_o[:_POOL, :], oh[:], ident[:])
                    jb = (i * _TOPK + t) * _P
                    if t % 2 == 0:
                        nc.scalar.copy(ohts[:, jb : jb + _P], ps_o[:_POOL, :])
                    else:
                        nc.vector.tensor_copy(ohts[:, jb : jb + _P], ps_o[:_POOL, :])

            # ---- phase B: pure gather stream, 30 matmuls per (tile, slot) ----
            for jidx in range(_NJ):
                st = stpool.tile([_P, _ROW], f16, tag="st")
                last = jidx == _NJ - 1
                for g in range(_NCH // 2):
                    ps_g = psg.tile([_P, 2 * _CHUNK], f32, space="PSUM")
                    for h in range(2):
                        c = 2 * g + h
                        nc.tensor.matmul(
                            ps_g[:, h * _CHUNK : (h + 1) * _CHUNK],
                            lhsT=ohts[:, jidx * _P : (jidx + 1) * _P],
                            rhs=p_h[:, c * _CHUNK : (c + 1) * _CHUNK],
                            start=True,
                            stop=True,
                        )
                    dst = st[:, 2 * g * _CHUNK : 2 * (g + 1) * _CHUNK]
                    if g % 2 == 0:
                        nc.scalar.copy(dst, ps_g[:])
                    else:
                        nc.vector.tensor_copy(dst, ps_g[:])
                    # last slot: stream the DMA out in thirds so the final
                    # drain after the last matmul is ~4 us instead of ~11
                    if last and g in (4, 9, 14):
                        a = (2 * (g - 4)) * _CHUNK
                        b = 2 * (g + 1) * _CHUNK
                        nc.sync.dma_start(
                            out=out_d[jidx, :, a:b], in_=st[:, a:b]
                        )
                # all output DMAs issue from the otherwise-idle Sync engine:
                # one HWDGE ring still spreads across all 16 SDMA engines,
                # and Scalar never stalls its copy stream on a DMA issue
                if not last:
                    nc.sync.dma_start(out=out_d[jidx, :, :], in_=st[:])

    nc.compile()
    return nc


_NC_CACHE = None


def _get_nc():
    global _NC_CACHE
    if _NC_CACHE is None:
        _install_axon_hooks()
        _NC_CACHE = build_bass()
    return _NC_CACHE


def kernel(x_query, x, K, p, layer_id, trace=False, tmpdir=None):
    from concourse.bass_utils import run_bass_kernel_spmd

    nc = _get_nc()

    x_query = np.ascontiguousarray(np.asarray(x_query, dtype=np.float32))
    K = np.ascontiguousarray(np.asarray(K, dtype=np.float32))
    p2 = np.ascontiguousarray(
        np.asarray(p, dtype=np.float32).reshape(_POOL, _ROW).astype(np.float16)
    )

    in_maps = []
    for c in range(_NCORES):
        in_maps.append(
            {
                "xq": x_query[c * _BSH : (c + 1) * _BSH],
                "kk": K,
                "pp": p2,
            }
        )

    kw = {}
    if trace:
        import concourse.bass_utils as bass_utils

        bass_utils.upload_artifacts = lambda d: d
        kw = {"trace": True, "tmpdir": tmpdir}
    res = run_bass_kernel_spmd(nc, in_maps, core_ids=list(range(_NCORES)), **kw)

    half = _PLEN // 2
    out = np.empty((2, _B, _TOPK * half, _D), dtype=np.float32)
    for c in range(_NCORES):
        arr = res.results[c]["out"].reshape(_NTILES, _TOPK, _P, 2, _HALF)
        # [tile, slot, row, half, elem] -> [half, tile, row, slot, elem]
        shard = arr.transpose(3, 0, 2, 1, 4).reshape(2, _BSH, _TOPK * half, _D)
        out[:, c * _BSH : (c + 1) * _BSH] = shard
    if trace:
        return out, res
    return out


# revision 29
# speedup vs baseline: 1.3254x; 1.0203x over previous
"""Trainium2 Bass kernel for L2P top-k prompt selection (topk_masking).

Reference computation:
    nk  = l2_normalize(K, axis=1)                 # [30, 768]
    sim = l2_normalize(x_query) @ nk.T            # [8192, 30]
    idx = top_k(sim, 5)                           # [8192, 5]
    sel = p[idx]                                  # [8192, 5, 20, 768]
    Ek  = sel[:, :, :10, :].reshape(B, 50, 768)
    Ev  = sel[:, :, 10:, :].reshape(B, 50, 768)
    out = stack([Ek, Ev])                         # [2, 8192, 50, 768]

Strategy (8 cores, data-parallel over batch):
  - query normalization is skipped: top-k ranking is invariant to positive
    per-row scaling of the query.
  - scores = xq @ nk.T on TensorE in fp32 (xq transposed on-chip via
    identity matmuls) — full precision so the selected indices match
    jax.lax.top_k exactly (near-ties matter).
  - top-5 via DVE max8/max_index (ties resolved to lowest index).
  - two phases: ALL tiles' scores/top-k/one-hot prep first, then one
    uninterrupted stream of 1200 one-hot gather matmuls. The PE column
    rate (1 col / 1.2 GHz cycle measured on this part) is the kernel
    floor, so the gather stream must never stall on cross-phase deps.
  - gather p rows via a single-pass fp16 one-hot matmul. The one-hot is
    exact in fp16 and PSUM accumulates fp32, so the gathered values are
    exactly fp16(p): rel err ~1.8e-4 L2, far inside the 2e-2 gate.
  - two 512-col matmuls fill a 2-bank PSUM pair; one 1024-col copy
    (alternating Scalar/Vector) drains it, halving copy overhead.
  - output is staged and written to HBM as fp16 in a contiguous dump
    layout [tile*slot, 128, 15360] (one DMA per (tile, slot), alternating
    the two HWDGE queues sync/scalar); the host reassembles/upcasts to
    the [2, B, 50, 768] fp32 result. This halves HBM write traffic vs
    fp32 output.
"""

import sys
import types

import numpy as np

_B = 8192
_DK = 768
_D = 768
_POOL = 30
_PLEN = 20
_TOPK = 5
_NCORES = 8
_BSH = _B // _NCORES          # 1024 batch rows per core
_P = 128
_NTILES = _BSH // _P          # 8 tiles of 128 rows
_ROW = _PLEN * _D             # 15360 floats per selected prompt
_HALF = _ROW // 2             # 7680 (Ek / Ev halves)
_CHUNK = 512
_NCH = _ROW // _CHUNK         # 30 psum chunks per (tile, slot)
_NJ = _NTILES * _TOPK         # 40 (tile, slot) output blocks


def _install_axon_hooks():
    """Make trace=True work under axon (profiling); harmless if absent."""
    if "antenv.axon_hooks" in sys.modules:
        return
    try:
        import trn_agent_boot.trn_boot as _tb

        hook = _tb._ntff_profile_via_ctypes("/opt/axon/libaxon_pjrt.so")
    except Exception:
        hook = None
    m = types.ModuleType("antenv.axon_hooks")
    m.get_axon_ntff_profile_hook = lambda: hook
    m.set_axon_ntff_profile_hook = lambda h: None
    sys.modules["antenv.axon_hooks"] = m


def build_bass():
    import concourse.bacc as bacc
    import concourse.mybir as mybir
    import concourse.tile as tile
    from concourse.masks import make_identity

    f32 = mybir.dt.float32
    f16 = mybir.dt.float16
    nc = bacc.Bacc(None, target_bir_lowering=False)

    xq_d = nc.dram_tensor("xq", [_BSH, _DK], f32, kind="ExternalInput")
    k_d = nc.dram_tensor("kk", [_POOL, _DK], f32, kind="ExternalInput")
    p_d = nc.dram_tensor("pp", [_POOL, _ROW], f16, kind="ExternalInput")
    out_d = nc.dram_tensor("out", [_NJ, _P, _ROW], f16, kind="ExternalOutput")

    with tile.TileContext(nc) as tc:
        with (
            tc.tile_pool(name="const", bufs=1) as cpool,
            tc.tile_pool(name="xq", bufs=2) as xqpool,
            tc.tile_pool(name="xqt", bufs=2) as xqtpool,
            tc.tile_pool(name="topk", bufs=2) as tkpool,
            tc.tile_pool(name="stage", bufs=4) as stpool,
            tc.tile_pool(name="ps_small", bufs=2, space="PSUM") as pss,
            tc.tile_pool(name="ps_gather", bufs=3, space="PSUM") as psg,
        ):
            # ---- constants and pool-side tensors ----
            ident = cpool.tile([_P, _P], f32)
            make_identity(nc, ident[:])

            iota_i = cpool.tile([_P, _POOL], mybir.dt.int32)
            nc.gpsimd.iota(iota_i[:], [[1, _POOL]], channel_multiplier=0)
            iota_f = cpool.tile([_P, _POOL], f32)
            nc.vector.tensor_copy(iota_f[:], iota_i[:])

            k_sb = cpool.tile([_POOL, _DK], f32)
            nc.gpsimd.dma_start(out=k_sb[:], in_=k_d[:])

            # all 40 transposed one-hots, built in phase A. One-hots are
            # PE-transposed in PAIRS ([128,64] -> [64,128]), so odd slots
            # land at partitions 32-61; the gather reads them there against
            # a second p copy at the same base (tile_position row 32).
            ohts = cpool.tile([32 + _POOL, _NJ * _P], f16)  # rows 0-29 / 32-61

            # K-normalization runs on ACT/DVE while PE does tile 0's xq
            # transposes; nkT lands on PE between those and the first scores
            nk = cpool.tile([_POOL, _DK], f32)
            ss = cpool.tile([_POOL, 1], f32)
            nrm = cpool.tile([_POOL, 1], f32)
            inv = cpool.tile([_POOL, 1], f32)
            nkt = cpool.tile([_P, 6 * _POOL], f32)
            p_h = cpool.tile([32 + _POOL, _ROW], f16)  # p at rows 0-29 AND 32-61

            # ---- phase A: scores + top-5 + transposed one-hots, all tiles ----
            for i in range(_NTILES):
                xq_sb = xqpool.tile([_P, _DK], f32)
                nc.gpsimd.dma_start(out=xq_sb[:], in_=xq_d[i * _P : (i + 1) * _P, :])

                # transpose xq tile -> xqT chunks [128f, 128b]
                xqt = xqtpool.tile([_P, _DK], f32)
                for j in range(6):
                    ps_t = pss.tile([_P, _P], f32, space="PSUM", tag="ps")
                    nc.tensor.transpose(
                        ps_t[:], xq_sb[:, j * _P : (j + 1) * _P], ident[:]
                    )
                    dst = xqt[:, j * _P : (j + 1) * _P]
                    if j % 2 == 0:
                        nc.scalar.copy(dst, ps_t[:])
                    else:
                        nc.vector.tensor_copy(dst, ps_t[:])

                if i == 0:
                    # normalize K rows: nk = K / ||K|| (ACT/DVE), then
                    # nkT [768, 30] as 6 chunks of [128, 30] on PE
                    nc.scalar.activation(
                        nk[:],
                        k_sb[:],
                        mybir.ActivationFunctionType.Square,
                        accum_out=ss[:],
                    )
                    nc.scalar.activation(
                        nrm[:], ss[:], mybir.ActivationFunctionType.Sqrt
                    )
                    nc.vector.reciprocal(inv[:], nrm[:])
                    nc.vector.tensor_scalar_mul(nk[:], k_sb[:], inv[:])
                    for j in range(6):
                        ps_t = pss.tile([_P, _P], f32, space="PSUM", tag="ps")
                        nc.tensor.transpose(
                            ps_t[:, :_POOL],
                            nk[:, j * _P : (j + 1) * _P],
                            ident[:_POOL, :_POOL],
                        )
                        nc.vector.tensor_copy(
                            nkt[:, j * _POOL : (j + 1) * _POOL], ps_t[:, :_POOL]
                        )
                    # p pool, pre-cast to fp16 on the host; queued after the
                    # first xq tiles so it doesn't delay phase A's start.
                    # Second copy at rows 32-61 serves odd-slot gathers.
                    nc.gpsimd.dma_start(out=p_h[:_POOL, :], in_=p_d[:])
                    nc.gpsimd.dma_start(out=p_h[32 : 32 + _POOL, :], in_=p_d[:])

                # scores [128b, 30] = sum_j xqT_j.T @ nkT_j
                ps_sc = pss.tile([_P, _P], f32, space="PSUM", tag="ps")
                for j in range(6):
                    nc.tensor.matmul(
                        ps_sc[:, :_POOL],
                        lhsT=xqt[:, j * _P : (j + 1) * _P],
                        rhs=nkt[:, j * _POOL : (j + 1) * _POOL],
                        start=(j == 0),
                        stop=(j == 5),
                    )
                sc = tkpool.tile([_P, _POOL], f32)
                nc.vector.tensor_copy(sc[:], ps_sc[:, :_POOL])

                # top-5 indices (ties -> lowest index, like jax.lax.top_k)
                mx = tkpool.tile([_P, 8], f32)
                mi = tkpool.tile([_P, 8], mybir.dt.uint32)
                nc.vector.max(mx[:], sc[:])
                nc.vector.max_index(mi[:], mx[:], sc[:])
                mif = tkpool.tile([_P, 8], f32)
                nc.vector.tensor_copy(mif[:], mi[:])

                # one-hots -> transposed fp16 blocks; slot pairs (0,1), (2,3)
                # share one [128,64]->[64,128] PE transpose, slot 4 goes solo
                for t0 in (0, 2, 4):
                    npair = 2 if t0 < 4 else 1
                    w = 32 * npair if npair == 2 else _POOL
                    # pad cols 30,31/62,63 transpose into PSUM rows that are
                    # never copied out, so they can stay uninitialized
                    oh = tkpool.tile([_P, 64], f32)
                    for h in range(npair):
                        nc.vector.tensor_tensor(
                            out=oh[:, 32 * h : 32 * h + _POOL],
                            in0=iota_f[:],
                            in1=mif[:, t0 + h : t0 + h + 1].to_broadcast(
                                [_P, _POOL]
                            ),
                            op=mybir.AluOpType.is_equal,
                        )
                    ps_o = pss.tile([_P, _P], f32, space="PSUM", tag="ps")
                    nc.tensor.transpose(ps_o[:w, :], oh[:, :w], ident[:])
                    for h in range(npair):
                        jb = (i * _TOPK + t0 + h) * _P
                        src = ps_o[32 * h : 32 * h + _POOL, :]
                        dst = ohts[32 * h : 32 * h + _POOL, jb : jb + _P]
                        if t0 + h == 0 or t0 + h == 3:
                            nc.scalar.copy(dst, src)
                        else:
                            nc.vector.tensor_copy(dst, src)

            # ---- phase B: pure gather stream, 30 matmuls per (tile, slot) ----
            for jidx in range(_NJ):
                st = stpool.tile([_P, _ROW], f16, tag="st")
                last = jidx == _NJ - 1
                # odd slots of each transpose pair live at partitions 32-61
                pb = 32 if (jidx % _TOPK) in (1, 3) else 0
                for g in range(_NCH // 2):
                    ps_g = psg.tile([_P, 2 * _CHUNK], f32, space="PSUM")
                    for h in range(2):
                        c = 2 * g + h
                        nc.tensor.matmul(
                            ps_g[:, h * _CHUNK : (h + 1) * _CHUNK],
                            lhsT=ohts[pb : pb + _POOL, jidx * _P : (jidx + 1) * _P],
                            rhs=p_h[pb : pb + _POOL, c * _CHUNK : (c + 1) * _CHUNK],
                            start=True,
                            stop=True,
                        )
                    dst = st[:, 2 * g * _CHUNK : 2 * (g + 1) * _CHUNK]
                    if g % 2 == 0:
                        nc.scalar.copy(dst, ps_g[:])
                    else:
                        nc.vector.tensor_copy(dst, ps_g[:])
                    # last slot: stream the DMA out in thirds so the final
                    # drain after the last matmul is ~4 us instead of ~11
                    if last and g in (4, 9, 14):
                        a = (2 * (g - 4)) * _CHUNK
                        b = 2 * (g + 1) * _CHUNK
                        nc.sync.dma_start(
                            out=out_d[jidx, :, a:b], in_=st[:, a:b]
                        )
                # all output DMAs issue from the otherwise-idle Sync engine:
                # one HWDGE ring still spreads across all 16 SDMA engines,
                # and Scalar never stalls its copy stream on a DMA issue
                if not last:
                    nc.sync.dma_start(out=out_d[jidx, :, :], in_=st[:])

    nc.compile()
    return nc


_NC_CACHE = None


def _get_nc():
    global _NC_CACHE
    if _NC_CACHE is None:
        _install_axon_hooks()
        _NC_CACHE = build_bass()
    return _NC_CACHE


def kernel(x_query, x, K, p, layer_id, trace=False, tmpdir=None):
    from concourse.bass_utils import run_bass_kernel_spmd

    nc = _get_nc()

    x_query = np.ascontiguousarray(np.asarray(x_query, dtype=np.float32))
    K = np.ascontiguousarray(np.asarray(K, dtype=np.float32))
    p2 = np.ascontiguousarray(
        np.asarray(p, dtype=np.float32).reshape(_POOL, _ROW).astype(np.float16)
    )

    in_maps = []
    for c in range(_NCORES):
        in_maps.append(
            {
                "xq": x_query[c * _BSH : (c + 1) * _BSH],
                "kk": K,
                "pp": p2,
            }
        )

    kw = {}
    if trace:
        import concourse.bass_utils as bass_utils

        bass_utils.upload_artifacts = lambda d: d
        kw = {"trace": True, "tmpdir": tmpdir}
    res = run_bass_kernel_spmd(nc, in_maps, core_ids=list(range(_NCORES)), **kw)

    half = _PLEN // 2
    out = np.empty((2, _B, _TOPK * half, _D), dtype=np.float32)
    for c in range(_NCORES):
        arr = res.results[c]["out"].reshape(_NTILES, _TOPK, _P, 2, _HALF)
        # [tile, slot, row, half, elem] -> [half, tile, row, slot, elem]
        shard = arr.transpose(3, 0, 2, 1, 4).reshape(2, _BSH, _TOPK * half, _D)
        out[:, c * _BSH : (c + 1) * _BSH] = shard
    if trace:
        return out, res
    return out
